# revision 29
# baseline (speedup 1.0000x reference)
"""AttentionPooler kernel for 8 Trainium2 NeuronCores.

Math rewrite (exact in real arithmetic):
  scores[b,n] = (hs[b,n,:] . q_eff) * H^-0.5  with q_eff = query[0] @ key_w
      (the q.key_b term is constant over n -> cancels in softmax)
  attn = softmax(scores) with masked tokens given weight 0
  out[b,:] = (sum_n attn[b,n] * hs[b,n,:]) @ value_w.T + value_b
      (value projection commutes with the attention average since sum attn = 1)

Sharding: data-parallel over batch, 8 batches per core. Each core streams its
50.3MB hidden_states slice once (HBM-bound). Per quarter-batch: scores on DVE
via fused scalar_tensor_tensor (dot product + accumulate), exp on ACT, masked
weights + row-sums on DVE, attention pooling on PE (float32r, the exp-weight
column as a 1-column stationary operand). Softmax denominators accumulate in
PSUM and normalization happens once on the pooled [8,768] result, fused with
the value_b bias add. The value projection runs once at the end on PE.
"""

import numpy as np

B, N, H = 64, 2048, 768
NCORES = 8
BL = B // NCORES  # batches per core
P = 128           # partitions
T = N // P        # token tiles per batch
HC = H // P       # h chunks

_CACHE = {}


def _build_nc():
    import concourse.tile as tile
    from concourse import bacc, mybir

    f32 = mybir.dt.float32
    f32r = mybir.dt.float32r
    bf16 = mybir.dt.bfloat16
    Alu = mybir.AluOpType
    Act = mybir.ActivationFunctionType

    def as32(ap):
        # view an f32r AP as plain f32 for vector/scalar-engine reads
        return ap.bitcast(f32)

    nc = bacc.Bacc(
        "TRN2",
        target_bir_lowering=False,
        debug=False,
        enable_asserts=False,
        num_devices=NCORES,
    )

    hs = nc.dram_tensor("hs", [BL, N, H], f32r, kind="ExternalInput").ap()
    mask = nc.dram_tensor("mask_f", [BL, P, T], f32, kind="ExternalInput").ap()
    q_cols = nc.dram_tensor("q_cols", [P, HC], bf16, kind="ExternalInput").ap()
    kw = nc.dram_tensor("kw", [HC, P, H], bf16, kind="ExternalInput").ap()
    vwt = nc.dram_tensor("vwt", [HC, P, H], f32r, kind="ExternalInput").ap()
    vb = nc.dram_tensor("vb", [1, H], f32r, kind="ExternalInput").ap()
    out = nc.dram_tensor("out", [BL, H], f32, kind="ExternalOutput").ap()

    inv_sqrt_h = float(1.0 / np.sqrt(H))

    with tile.TileContext(nc) as tc:
        from contextlib import ExitStack

        with ExitStack() as ctx:
            const = ctx.enter_context(tc.tile_pool(name="const", bufs=1))
            score = ctx.enter_context(tc.tile_pool(name="score", bufs=2))
            scratch = ctx.enter_context(tc.tile_pool(name="scratch", bufs=2))
            den_pool = ctx.enter_context(
                tc.tile_pool(name="den_psum", bufs=1, space="PSUM")
            )

            # ---- constants (qc/kw first: the q_eff chain gates the scores)
            qc_sb = const.tile([P, HC], bf16)
            nc.sync.dma_start(out=qc_sb, in_=q_cols)
            mk_all = const.tile([P, BL, T], f32)
            vb_sb = const.tile([1, H], f32r)
            ones_col = const.tile([P, 1], f32)
            nc.vector.memset(ones_col, 1.0)
            ones_row = const.tile([1, P], f32)
            nc.vector.memset(ones_row, 1.0)

            qe_sb = const.tile([1, H], f32)
            qrep_sb = const.tile([P, H], f32)
            pooledT_sb = const.tile([P, HC, BL], f32r)
            den_sb = const.tile([1, BL], f32)
            vb_rep = const.tile([BL, H], f32)
            denT_sb = const.tile([BL, 1], f32)
            recip_sb = const.tile([BL, 1], f32)
            out_sb = const.tile([BL, H], f32)
            vwt_sb = const.tile([P, HC, H], f32r)

            den_ps = den_pool.tile([1, BL], f32)

            # ---- q_eff = (query @ key_w) * H^-0.5, replicated to 128
            # partitions. kw lives in a scoped pool released before the big
            # hs group pool is created (SBUF headroom for triple buffering).
            with tc.tile_pool(name="kw_pool", bufs=1) as kw_pool:
                kw_sb = kw_pool.tile([P, HC, H], bf16)
                with tc.tile_pool(name="setup_psum", bufs=1, space="PSUM") as sp:
                    qe_ps = sp.tile([1, H], f32)
                    for c in range(HC):
                        nc.sync.dma_start(out=kw_sb[:, c, :], in_=kw[c])
                        nc.tensor.matmul(
                            qe_ps[:, 0:512],
                            lhsT=qc_sb[:, c : c + 1],
                            rhs=kw_sb[:, c, 0:512],
                            start=(c == 0),
                            stop=(c == HC - 1),
                        )
                        nc.tensor.matmul(
                            qe_ps[:, 512:H],
                            lhsT=qc_sb[:, c : c + 1],
                            rhs=kw_sb[:, c, 512:H],
                            start=(c == 0),
                            stop=(c == HC - 1),
                        )
                    nc.sync.dma_start(
                        out=mk_all, in_=mask.rearrange("b p t -> p b t")
                    )
                    nc.sync.dma_start(out=vb_sb, in_=vb)
                    nc.scalar.activation(
                        out=qe_sb, in_=qe_ps, func=Act.Copy, scale=inv_sqrt_h
                    )
                    qrep_ps = sp.tile([P, H], f32)
                    nc.tensor.matmul(
                        qrep_ps[:, 0:512], lhsT=ones_row, rhs=qe_sb[:, 0:512],
                        start=True, stop=True,
                    )
                    nc.tensor.matmul(
                        qrep_ps[:, 512:H], lhsT=ones_row, rhs=qe_sb[:, 512:H],
                        start=True, stop=True,
                    )
                    nc.vector.tensor_copy(out=qrep_sb, in_=qrep_ps)
                    vbr_ps = sp.tile([BL, H], f32)
                    nc.tensor.matmul(
                        vbr_ps[:, 0:512], lhsT=ones_row[0:1, 0:BL],
                        rhs=as32(vb_sb[:, 0:512]), start=True, stop=True,
                    )
                    nc.tensor.matmul(
                        vbr_ps[:, 512:H], lhsT=ones_row[0:1, 0:BL],
                        rhs=as32(vb_sb[:, 512:H]), start=True, stop=True,
                    )
                    nc.scalar.activation(out=vb_rep, in_=vbr_ps, func=Act.Copy)

            group = ctx.enter_context(tc.tile_pool(name="group", bufs=3))

            # ---- main loop: stream hidden_states once ----
            with tc.tile_pool(name="pool_psum", bufs=2, space="PSUM") as pp:
                for b in range(BL):
                    if b == 1:
                        # value_w^T is only needed in the finale; load it
                        # mid-stream, off the startup critical path
                        nc.sync.dma_start(
                            out=vwt_sb, in_=vwt.rearrange("c p h -> p c h")
                        )
                    hs_g = group.tile([P, T, H], f32r, tag="hs_g")
                    pool_ps = pp.tile([1, H], f32, tag="pooled")
                    # process in chunks: exp is elementwise and the softmax
                    # denominator is accumulated separately, so pooling follows
                    # each chunk's scores with no full-batch barrier. The last
                    # batch tapers to 1-tile chunks so the final dependency
                    # chain after the last DMA is short.
                    chunks = [4, 4, 4, 4] if b < BL - 1 else [4, 4, 4, 2, 1, 1]
                    t = 0
                    for ci, cw in enumerate(chunks):
                        t0c = t
                        s_q = score.tile([P, cw], f32, tag="s_q", bufs=3)
                        for t4 in range(cw):
                            t = t0c + t4
                            nc.sync.dma_start(
                                out=hs_g[:, t, :], in_=hs[b, t * P : (t + 1) * P, :]
                            )
                            nc.vector.scalar_tensor_tensor(
                                out=scratch.tile([P, H], f32, tag="prod", name="prod"),
                                in0=as32(hs_g[:, t, :]),
                                scalar=1.0,
                                in1=qrep_sb,
                                op0=Alu.mult,
                                op1=Alu.mult,
                                accum_out=s_q[:, t4 : t4 + 1],
                            )
                        t = t0c + cw
                        # weights: w = exp(s) * mask ; row-sums -> ps_q
                        w_q = score.tile([P, cw], f32, tag="w_q", bufs=3)
                        nc.scalar.activation(out=w_q, in_=s_q, func=Act.Exp)
                        wm_q = score.tile([P, cw], f32r, tag="wm_q", bufs=3)
                        ps_q = score.tile([P, 1], f32, tag="ps_q", bufs=3)
                        nc.vector.scalar_tensor_tensor(
                            out=wm_q,
                            in0=w_q,
                            scalar=1.0,
                            in1=mk_all[:, b, t0c : t0c + cw],
                            op0=Alu.mult,
                            op1=Alu.mult,
                            accum_out=ps_q,
                        )
                        # denominator: accumulate sum over partitions -> den_ps[0, b]
                        nc.tensor.matmul(
                            den_ps[:, b : b + 1], lhsT=ps_q, rhs=ones_col,
                            start=(ci == 0), stop=(ci == len(chunks) - 1),
                        )
                        # pooling: pooled += sum_t w_t * hs_t
                        for t4 in range(cw):
                            tt = t0c + t4
                            nc.tensor.matmul(
                                pool_ps[:, 0:512],
                                lhsT=wm_q[:, t4 : t4 + 1],
                                rhs=hs_g[:, tt, 0:512],
                                start=(tt == 0),
                                stop=(tt == T - 1),
                            )
                            nc.tensor.matmul(
                                pool_ps[:, 512:H],
                                lhsT=wm_q[:, t4 : t4 + 1],
                                rhs=hs_g[:, tt, 512:H],
                                start=(tt == 0),
                                stop=(tt == T - 1),
                            )
                    if b == BL - 1:
                        # denominator chain: overlaps the last batch's pooling
                        nc.vector.tensor_copy(out=den_sb, in_=den_ps)
                        denT_ps = den_pool.tile([BL, 1], f32, tag="denT")
                        nc.tensor.matmul(
                            denT_ps, lhsT=den_sb, rhs=ones_row[0:1, 0:1],
                            start=True, stop=True,
                        )
                        nc.vector.tensor_copy(out=denT_sb, in_=denT_ps)
                        nc.vector.reciprocal(out=recip_sb, in_=denT_sb)
                    # evacuate pooled [1,768] to an sbuf row, transpose into
                    # column b of pooledT (PE + ACT only: the DVE queue stays
                    # free for next batch's scores)
                    pooled_row = score.tile([1, H], f32, tag="pooled_row")
                    nc.scalar.activation(
                        out=pooled_row, in_=pool_ps, func=Act.Copy
                    )
                    tp_ps = pp.tile([P, HC], f32, tag="tp")
                    for c in range(HC):
                        nc.tensor.matmul(
                            tp_ps[:, c : c + 1],
                            lhsT=pooled_row[0:1, c * P : (c + 1) * P],
                            rhs=ones_row[0:1, 0:1],
                            start=True,
                            stop=True,
                        )
                    nc.scalar.activation(
                        out=pooledT_sb[:, :, b], in_=tp_ps, func=Act.Copy
                    )

            # ---- finale: denominators, project, normalize ----
            with tc.tile_pool(name="fin_psum", bufs=1, space="PSUM") as fp:
                proj_ps = fp.tile([BL, H], f32, tag="proj")
                for c in range(HC):
                    nc.tensor.matmul(
                        proj_ps[:, 0:512],
                        lhsT=pooledT_sb[:, c, :],
                        rhs=vwt_sb[:, c, 0:512],
                        start=(c == 0),
                        stop=(c == HC - 1),
                    )
                    nc.tensor.matmul(
                        proj_ps[:, 512:H],
                        lhsT=pooledT_sb[:, c, :],
                        rhs=vwt_sb[:, c, 512:H],
                        start=(c == 0),
                        stop=(c == HC - 1),
                    )
                # out = proj/den + value_b in one fused DVE op
                nc.vector.scalar_tensor_tensor(
                    out=out_sb,
                    in0=proj_ps,
                    scalar=recip_sb,
                    in1=vb_rep,
                    op0=Alu.mult,
                    op1=Alu.add,
                )
                nc.sync.dma_start(out=out, in_=out_sb)

    nc.compile()
    return nc


def _get_nc():
    if "nc" not in _CACHE:
        _CACHE["nc"] = _build_nc()
    return _CACHE["nc"]


def _prep_in_maps(hidden_states, attention_mask, query, key_w, key_b, value_w, value_b):
    hs = np.ascontiguousarray(np.asarray(hidden_states, dtype=np.float32))
    mask = np.asarray(attention_mask)
    q = np.asarray(query, dtype=np.float32)
    kw_np = np.asarray(key_w, dtype=np.float32)
    vw_np = np.asarray(value_w, dtype=np.float32)
    vb_np = np.asarray(value_b, dtype=np.float32)

    # mask[b, n] with n = t*128 + p  ->  [b, p, t] float32
    mask_f = np.ascontiguousarray(
        mask.reshape(B, T, P).transpose(0, 2, 1).astype(np.float32)
    )
    import ml_dtypes

    q_cols = np.ascontiguousarray(q.reshape(HC, P).T).astype(ml_dtypes.bfloat16)
    kw3 = np.ascontiguousarray(kw_np.reshape(HC, P, H)).astype(ml_dtypes.bfloat16)
    vwt3 = np.ascontiguousarray(vw_np.T.reshape(HC, P, H))
    vb2 = np.ascontiguousarray(vb_np.reshape(1, H))

    in_maps = []
    for c in range(NCORES):
        in_maps.append(
            {
                "hs": np.ascontiguousarray(hs[c * BL : (c + 1) * BL]),
                "mask_f": np.ascontiguousarray(mask_f[c * BL : (c + 1) * BL]),
                "q_cols": q_cols,
                "kw": kw3,
                "vwt": vwt3,
                "vb": vb2,
            }
        )
    return in_maps


def _run(in_maps, trace=False, **kwargs):
    from concourse.bass_utils import run_bass_kernel_spmd

    nc = _get_nc()
    res = run_bass_kernel_spmd(
        nc, in_maps, core_ids=list(range(NCORES)), trace=trace, **kwargs
    )
    out = np.concatenate([r["out"] for r in res.results], axis=0)
    return out, res


def kernel(**inputs):
    in_maps = _prep_in_maps(**inputs)
    out, _ = _run(in_maps, trace=False)
    return out
